# revision 17
# baseline (speedup 1.0000x reference)
"""Trainium2 Bass kernel for per-combination linear encoder (embedding lookup).

Computes z = y * w[idx] + b[idx] where idx = t*1024 + x @ [512,256,...,1]
for x in {0,1}^[N,10], t in {0,1}^[N,1], over a 2048-entry (w,b) table.

Sharding: data-parallel over the batch axis across 8 NeuronCores; the
tiny (w,b) table is replicated to every core.

Each core splits its rows between two pipelines that run concurrently:

GATHER PATH (Rg rows, GPSIMD-bound at ~3.4ns/row):
  DVE computes idx in fp16 (exact <=2047), GPSIMD ap_gather fetches
  packed (w,b) bf16 pairs (16x-replicated per Q7 core), PE un-wraps via
  16 accumulating diagonal-mask bf16 matmuls, DVE does the fp32 FMA.

SIDE PATH (Rs rows, PE/DVE/ACT pipeline, column-major layout):
  Split idx = 32*hi + lo (hi in [0,64), lo in [0,32)). Using step
  functions S[c] = [hi >= c+1] (exact 0/1 via one ACT sigmoid pass,
  sigma(40u+20) saturates exactly in fp16 for integer u):
    W2[hi, l] = sum_c S[c] * G[c, l]      (G = first differences of the
                                           table along hi, G[63] = row 0)
  and Abel summation to avoid materializing the lo one-hot:
    sum_l T[l]*[lo==l] = sum_l (T[l]-T[l-1]) * [lo >= l]
  so with Gd = diff of G along lo, one PE matmul (B1) produces
  DeltaT[l] = T[l]-T[l-1] directly, DVE multiplies by the lo-step matrix
  (y-scaled on the w-plane), and a final PE matmul (B2) of ones reduces
  to z, accumulating 32 subtiles into distinct PSUM partitions.

  Per 512-row subtile: A-matmul (bits->step args), ACT sigmoid (steps),
  DVE y-scale, B1-matmul, DVE product, B2-matmul. 3 PE cols/row total.
"""

import numpy as np
import ml_dtypes

import concourse.bacc as bacc
import concourse.mybir as mybir
from concourse.tile import TileContext
from concourse.bass_utils import run_bass_kernel_spmd

M = 8            # NeuronCores
P = 128          # SBUF partitions
D = 11           # [t | x] covariate bits
C = 2048         # table entries
F32 = mybir.dt.float32
F16 = mybir.dt.float16
BF16 = mybir.dt.bfloat16
I16 = mybir.dt.int16

# ---- row split per core ----
G_SCHED = (264, 264, 265, 265)   # gather-path rows-per-partition schedule
RPPG = sum(G_SCHED)              # 674
RG = P * RPPG                    # 135424 gather-path rows
SF = 512                         # side-path subtile rows (columns)
ZC = 32                          # subtiles accumulated per z-psum cycle
ZO = 7                           # z cycles
RS = ZO * ZC * SF                # 114688 side-path rows
R = RG + RS                      # 250112 rows per core
SUBS_PER_ROUND = ZO * ZC // len(G_SCHED)   # 56

_CACHE = {}


def _build_program():
    nc = bacc.Bacc("TRN2", target_bir_lowering=False, debug=False, num_devices=M)

    # gather-path tensors
    xt = nc.dram_tensor("xt", [RG, D], F16, kind="ExternalInput")
    y = nc.dram_tensor("y", [RG], F32, kind="ExternalInput")
    wb = nc.dram_tensor("wb", [P, C], F32, kind="ExternalInput")
    pw = nc.dram_tensor("pw", [P, D], F16, kind="ExternalInput")
    mk = nc.dram_tensor("mk", [P, 16 * P], BF16, kind="ExternalInput")
    z = nc.dram_tensor("z", [RG], F32, kind="ExternalOutput")
    # side-path tensors
    sb = nc.dram_tensor("sb", [12, RS], BF16, kind="ExternalInput")
    sy = nc.dram_tensor("sy", [32, RS], F16, kind="ExternalInput")
    la = nc.dram_tensor("la", [12, 128], BF16, kind="ExternalInput")
    lb1 = nc.dram_tensor("lb1", [P, 64], F16, kind="ExternalInput")
    lb2 = nc.dram_tensor("lb2", [64, ZC * 32], F16, kind="ExternalInput")
    c20 = nc.dram_tensor("c20", [P, 1], F32, kind="ExternalInput")
    zs = nc.dram_tensor("zs", [ZO * ZC, SF], F32, kind="ExternalOutput")

    x3 = xt.ap().rearrange("(pp r) d -> pp (r d)", pp=P)   # [P, RPPG*D]
    y2 = y.ap().rearrange("(pp r) -> pp r", pp=P)
    z2 = z.ap().rearrange("(pp r) -> pp r", pp=P)

    with TileContext(nc) as tc:
        with (
            tc.tile_pool(name="const", bufs=1) as cpool,
            tc.tile_pool(name="sb", bufs=4) as pool,
            tc.tile_pool(name="gat", bufs=4) as gpool,
            tc.tile_pool(name="sp", bufs=3) as spool,
            tc.tile_pool(name="gps", bufs=1, space="PSUM") as gppool,
            tc.tile_pool(name="apsp", bufs=2, space="PSUM") as appool,
            tc.tile_pool(name="tpsp", bufs=2, space="PSUM") as tppool,
            tc.tile_pool(name="zp", bufs=2, space="PSUM") as zpool,
        ):
            wb_t = cpool.tile([P, C], F32)
            nc.sync.dma_start(out=wb_t[:], in_=wb[:, :])
            pw_t = cpool.tile([P, D], F16)
            nc.sync.dma_start(out=pw_t[:], in_=pw[:, :])
            mk_t = cpool.tile([P, 16 * P], BF16)
            nc.sync.dma_start(out=mk_t[:], in_=mk[:, :])
            la_t = cpool.tile([12, 128], BF16)
            nc.sync.dma_start(out=la_t[:], in_=la[:, :])
            lb1_t = cpool.tile([P, 64], F16)
            nc.sync.dma_start(out=lb1_t[:], in_=lb1[:, :])
            lb2_t = cpool.tile([64, ZC * 32], F16)
            nc.sync.dma_start(out=lb2_t[:], in_=lb2[:, :])
            c20_t = cpool.tile([P, 1], F32)
            nc.sync.dma_start(out=c20_t[:], in_=c20[:, :])

            # ---------------- gather path helpers ----------------
            def g_load_idx(i, off, B):
                xtt = pool.tile([P, B * D], F16, tag="x")
                nc.sync.dma_start(out=xtt[:], in_=x3[:, off * D:(off + B) * D])
                yt = pool.tile([P, B], F32, tag="y")
                nc.sync.dma_start(out=yt[:], in_=y2[:, off:off + B])
                xv = xtt[:].rearrange("p (b d) -> p b d", d=D)
                nc.vector.tensor_tensor(
                    out=xv, in0=xv,
                    in1=pw_t[:].unsqueeze(1).broadcast_to([P, B, D]),
                    op=mybir.AluOpType.mult,
                )
                idxf = pool.tile([P, B], F16, tag="idxf")
                with nc.allow_low_precision(
                    reason="index accumulation is integer-exact in fp16 (<=2047)"
                ):
                    nc.vector.tensor_reduce(
                        out=idxf[:], in_=xv, axis=mybir.AxisListType.X,
                        op=mybir.AluOpType.add,
                    )
                idx16 = pool.tile([P, B], I16, tag="idx16")
                nc.vector.tensor_copy(out=idx16[:], in_=idxf[:])
                return yt, idx16

            def g_gather(idx16, B):
                og = gpool.tile([P, 16 * B], F32, tag="og")
                nc.gpsimd.ap_gather(
                    out_ap=og[:].rearrange("p (j e) -> p j e", e=1),
                    in_ap=wb_t[:].rearrange("p (c e) -> p c e", e=1),
                    idxs_ap=idx16[:],
                    channels=P, num_elems=C, d=1, num_idxs=16 * B,
                )
                return og

            def g_unwrap(og, yt, off, B):
                og3 = og[:].bitcast(BF16).rearrange("p (c s) -> p c s", s=32)
                psw = gppool.tile([P, B], F32, tag="psw")
                psb = gppool.tile([P, B], F32, tag="psb")
                for q in range(16):
                    nc.tensor.matmul(
                        out=psw[:], lhsT=mk_t[:, q * P:(q + 1) * P],
                        rhs=og3[:, :, 2 * q], start=(q == 0), stop=(q == 15),
                    )
                for q in range(16):
                    nc.tensor.matmul(
                        out=psb[:], lhsT=mk_t[:, q * P:(q + 1) * P],
                        rhs=og3[:, :, 2 * q + 1], start=(q == 0), stop=(q == 15),
                    )
                zt = pool.tile([P, B], F32, tag="z")
                nc.vector.tensor_tensor(
                    out=zt[:], in0=yt[:], in1=psw[:], op=mybir.AluOpType.mult
                )
                nc.vector.tensor_tensor(
                    out=zt[:], in0=zt[:], in1=psb[:], op=mybir.AluOpType.add
                )
                nc.sync.dma_start(out=z2[:, off:off + B], in_=zt[:])

            # ---------------- side path (skew-2 software pipeline) ----------------
            SBAT = 8                        # subtiles per batched DMA
            zstate = {"zps": None}
            st = {"bits": {}, "yr": {}, "sg": {}, "pr": {}, "aps": {}, "tps": {}}

            def s_batch_dma(k):
                c0 = k * SBAT * SF
                bb = spool.tile([12, SBAT * SF], BF16, tag="bits", name="bb")
                nc.sync.dma_start(out=bb[:], in_=sb[:, c0:c0 + SBAT * SF])
                yy = spool.tile([32, SBAT * SF], F16, tag="yr", name="yy")
                nc.sync.dma_start(out=yy[:], in_=sy[:, c0:c0 + SBAT * SF])
                st["bits"][k] = bb
                st["yr"][k] = yy

            def s_stage_a(s):
                k, r = divmod(s, SBAT)
                if r == 0 and k + 1 < ZO * ZC // SBAT:
                    s_batch_dma(k + 1)
                bits = st["bits"][k][:, r * SF:(r + 1) * SF]
                aps = appool.tile([P, SF], F32, tag="aps", name="aps")
                nc.tensor.matmul(out=aps[:], lhsT=la_t[:], rhs=bits,
                                 start=True, stop=True)
                st["aps"][s] = aps

            def s_stage_sg(s):
                k, r = divmod(s, SBAT)
                # steps: exact 0/1 for integer args (sigma(+->=20) saturates)
                sg = spool.tile([P, SF], F16, tag="sg", name="sg")
                nc.scalar.activation(
                    out=sg[:], in_=st["aps"].pop(s)[:],
                    func=mybir.ActivationFunctionType.Sigmoid,
                    bias=c20_t[:], scale=40.0,
                )
                # scale w-plane lo-steps by y
                nc.vector.tensor_tensor(
                    out=sg[0:32, :], in0=sg[0:32, :],
                    in1=st["yr"][k][:, r * SF:(r + 1) * SF],
                    op=mybir.AluOpType.mult,
                )
                st["sg"][s] = sg

            def s_stage_b1(s):
                sg = st["sg"][s]
                tps = tppool.tile([64, SF], F32, tag="tps", name="tps")
                nc.tensor.matmul(out=tps[:], lhsT=lb1_t[64:128, :],
                                 rhs=sg[64:128, :], start=True, stop=True)
                st["tps"][s] = tps

            def s_stage_pr(s):
                sg = st["sg"].pop(s)
                pr = spool.tile([64, SF], F16, tag="pr", name="pr")
                nc.vector.tensor_tensor(
                    out=pr[:], in0=st["tps"].pop(s)[:], in1=sg[0:64, :],
                    op=mybir.AluOpType.mult,
                )
                st["pr"][s] = pr

            def s_stage_b2(s):
                o, j = divmod(s, ZC)
                if j == 0:
                    zstate["zps"] = zpool.tile([ZC, SF], F32, tag="zps", name="zps")
                nc.tensor.matmul(
                    out=zstate["zps"][:], lhsT=lb2_t[:, 32 * j:32 * (j + 1)],
                    rhs=st["pr"].pop(s)[:], start=(j == 0), stop=(j == ZC - 1),
                )
                if j == ZC - 1:
                    zsb = spool.tile([ZC, SF], F32, tag="zsb", name="zsb")
                    nc.vector.tensor_copy(out=zsb[:], in_=zstate["zps"][:])
                    nc.sync.dma_start(out=zs[o * ZC:(o + 1) * ZC, :], in_=zsb[:])

            def s_subtile(it, total):
                # iteration it: A(it), B1(it-1), B2(it-2) keep PE dense
                if it < total:
                    s_stage_a(it)
                if 0 <= it - 1 < total:
                    s_stage_b1(it - 1)
                if 0 <= it - 2 < total:
                    s_stage_b2(it - 2)
                if it < total:
                    s_stage_sg(it)
                if 0 <= it - 1 < total:
                    s_stage_pr(it - 1)

            # ---------------- emission: interleave rounds ----------------
            nrounds = len(G_SCHED)
            goff = [0]
            for B in G_SCHED:
                goff.append(goff[-1] + B)
            gpre = [g_load_idx(i, goff[i], G_SCHED[i]) for i in range(nrounds)]
            ogs = [g_gather(gpre[i][1], G_SCHED[i]) for i in range(nrounds)]
            s_batch_dma(0)
            total = ZO * ZC
            sctr = 0
            for i in range(nrounds):
                for _ in range(SUBS_PER_ROUND):
                    s_subtile(sctr, total)
                    sctr += 1
                if i >= 1:
                    with tc.tile_wait_until(0.13 + 0.035 * (i - 1)):
                        g_unwrap(ogs[i - 1], gpre[i - 1][0], goff[i - 1],
                                 G_SCHED[i - 1])
            while sctr < total + 2:
                s_subtile(sctr, total)
                sctr += 1
            with tc.tile_wait_until(0.13 + 0.035 * (len(G_SCHED) - 1)):
                g_unwrap(ogs[-1], gpre[-1][0], goff[len(G_SCHED) - 1],
                         G_SCHED[-1])

    nc.compile()
    return nc


def _get_program():
    if "nc" not in _CACHE:
        _CACHE["nc"] = _build_program()
    return _CACHE["nc"]


def _host_prep(x, t, y, w, b):
    N = x.shape[0]
    npad = M * R - N
    assert npad >= 0
    f32 = np.float32
    f16 = np.float16
    bf16 = ml_dtypes.bfloat16

    # full [t|x] bit matrix [M*R, 11] fp16 (values 0/1 exact)
    bits = np.zeros((M * R, D), f16)
    bits[:N, 0] = np.asarray(t, f32).reshape(-1)
    bits[:N, 1:] = np.asarray(x, f32)
    yfull = np.concatenate([np.asarray(y, f32).reshape(-1), np.zeros(npad, f32)])

    bits_c = bits.reshape(M, R, D)
    y_c = yfull.reshape(M, R)

    # gather path: first RG rows of each shard, row-major per partition
    xtp = np.ascontiguousarray(bits_c[:, :RG, :])                    # [M, RG, 11]
    yp = np.ascontiguousarray(y_c[:, :RG])                           # [M, RG]
    # side path: remaining RS rows, column-major [12, RS] with ones row
    sbp = np.empty((M, 12, RS), ml_dtypes.bfloat16)
    sbp[:, :D, :] = bits_c[:, RG:, :].transpose(0, 2, 1)
    sbp[:, D, :] = 1.0
    syp = np.ascontiguousarray(
        np.broadcast_to(y_c[:, None, RG:], (M, 32, RS)).astype(f16)
    )

    # constants
    wbits = np.asarray(w, f32).astype(bf16).view(np.uint16).astype(np.uint32)
    bbits = np.asarray(b, f32).astype(bf16).view(np.uint16).astype(np.uint32)
    wbi = (wbits | (bbits << 16)).view(f32)
    wb_rep = np.ascontiguousarray(np.tile(wbi[None, :], (P, 1)))
    pw_rep = np.ascontiguousarray(
        np.tile((2.0 ** np.arange(D - 1, -1, -1)).astype(f32)[None, :], (P, 1))
    ).astype(f16)
    mk_host = np.zeros((P, 16 * P), f32)
    for k in range(P):
        mk_host[k, (k % 16) * P + k] = 1.0
    mk_host = mk_host.astype(bf16)

    # side-path weights
    la_h = np.zeros((12, 128), f32)
    hi_pw = np.array([32, 16, 8, 4, 2, 1], f32)       # t, x0..x4
    lo_pw = np.array([16, 8, 4, 2, 1], f32)           # x5..x9
    for m in range(64):
        l = m % 32
        la_h[6:11, m] = lo_pw
        la_h[11, m] = -float(l)
    for c in range(63):
        la_h[0:6, 64 + c] = hi_pw
        la_h[11, 64 + c] = -float(c + 1)
    la_h[11, 127] = 1.0

    W2 = np.asarray(w, f32).reshape(64, 32)
    B2t = np.asarray(b, f32).reshape(64, 32)

    def dd(T):
        G = np.empty_like(T)
        G[:63] = T[1:] - T[:-1]
        G[63] = T[0]
        Gd = np.empty_like(G)
        Gd[:, 0] = G[:, 0]
        Gd[:, 1:] = G[:, 1:] - G[:, :-1]
        return Gd

    lb1_h = np.zeros((P, 64), f32)
    lb1_h[64:, :32] = dd(W2)
    lb1_h[64:, 32:] = dd(B2t)

    lb2_h = np.zeros((64, ZC * 32), f32)
    for j in range(ZC):
        lb2_h[:, 32 * j + j] = 1.0

    in_maps = []
    for i in range(M):
        in_maps.append({
            "xt": xtp[i], "y": yp[i], "wb": wb_rep, "pw": pw_rep,
            "mk": mk_host, "sb": np.ascontiguousarray(sbp[i]),
            "sy": syp[i], "la": la_h.astype(ml_dtypes.bfloat16),
            "lb1": lb1_h.astype(f16), "lb2": lb2_h.astype(f16),
            "c20": np.full((P, 1), 20.0, f32),
        })
    return in_maps, npad


def kernel(x, t, y, w, b, trace=False):
    N = x.shape[0]
    in_maps, npad = _host_prep(x, t, y, w, b)
    nc = _get_program()
    res = run_bass_kernel_spmd(nc, in_maps, core_ids=list(range(M)), trace=trace)
    out = np.empty((M, R), np.float32)
    for i in range(M):
        out[i, :RG] = res.results[i]["z"]
        out[i, RG:] = res.results[i]["zs"].reshape(-1)
    zfull = out.reshape(-1)[:N].reshape(N, 1).astype(np.float32)
    if trace:
        return zfull, res
    return zfull


# revision 18
# speedup vs baseline: 1.0025x; 1.0025x over previous
"""Trainium2 Bass kernel for per-combination linear encoder (embedding lookup).

Computes z = y * w[idx] + b[idx] where idx = t*1024 + x @ [512,256,...,1]
for x in {0,1}^[N,10], t in {0,1}^[N,1], over a 2048-entry (w,b) table.

Sharding: data-parallel over the batch axis across 8 NeuronCores; the
tiny (w,b) table is replicated to every core.

Each core splits its rows between two pipelines that run concurrently:

GATHER PATH (Rg rows, GPSIMD-bound at ~3.4ns/row):
  DVE computes idx in fp16 (exact <=2047), GPSIMD ap_gather fetches
  packed (w,b) bf16 pairs (16x-replicated per Q7 core), PE un-wraps via
  16 accumulating diagonal-mask bf16 matmuls, DVE does the fp32 FMA.

SIDE PATH (Rs rows, PE/DVE/ACT pipeline, column-major layout):
  Split idx = 32*hi + lo (hi in [0,64), lo in [0,32)). Using step
  functions S[c] = [hi >= c+1] (exact 0/1 via one ACT sigmoid pass,
  sigma(40u+20) saturates exactly in fp16 for integer u):
    W2[hi, l] = sum_c S[c] * G[c, l]      (G = first differences of the
                                           table along hi, G[63] = row 0)
  and Abel summation to avoid materializing the lo one-hot:
    sum_l T[l]*[lo==l] = sum_l (T[l]-T[l-1]) * [lo >= l]
  so with Gd = diff of G along lo, one PE matmul (B1) produces
  DeltaT[l] = T[l]-T[l-1] directly, DVE multiplies by the lo-step matrix
  (y-scaled on the w-plane), and a final PE matmul (B2) of ones reduces
  to z, accumulating 32 subtiles into distinct PSUM partitions.

  Per 512-row subtile: A-matmul (bits->step args), ACT sigmoid (steps),
  DVE y-scale, B1-matmul, DVE product, B2-matmul. 3 PE cols/row total.
"""

import numpy as np
import ml_dtypes

import concourse.bacc as bacc
import concourse.mybir as mybir
from concourse.tile import TileContext
from concourse.bass_utils import run_bass_kernel_spmd

M = 8            # NeuronCores
P = 128          # SBUF partitions
D = 11           # [t | x] covariate bits
C = 2048         # table entries
F32 = mybir.dt.float32
F16 = mybir.dt.float16
BF16 = mybir.dt.bfloat16
I16 = mybir.dt.int16

# ---- row split per core ----
G_SCHED = (264, 264, 265, 265)   # gather-path rows-per-partition schedule
RPPG = sum(G_SCHED)              # 674
RG = P * RPPG                    # 135424 gather-path rows
SF = 512                         # side-path subtile rows (columns)
ZC = 32                          # subtiles accumulated per z-psum cycle
ZO = 7                           # z cycles
RS = ZO * ZC * SF                # 114688 side-path rows
R = RG + RS                      # 250112 rows per core
SUBS_PER_ROUND = ZO * ZC // len(G_SCHED)   # 56

_CACHE = {}


def _build_program():
    nc = bacc.Bacc("TRN2", target_bir_lowering=False, debug=False, num_devices=M)

    # gather-path tensors
    xt = nc.dram_tensor("xt", [RG, D], F16, kind="ExternalInput")
    y = nc.dram_tensor("y", [RG], F32, kind="ExternalInput")
    wb = nc.dram_tensor("wb", [P, C], F32, kind="ExternalInput")
    pw = nc.dram_tensor("pw", [P, D], F16, kind="ExternalInput")
    mk = nc.dram_tensor("mk", [P, 16 * P], BF16, kind="ExternalInput")
    z = nc.dram_tensor("z", [RG], F32, kind="ExternalOutput")
    # side-path tensors
    sb = nc.dram_tensor("sb", [12, RS], BF16, kind="ExternalInput")
    sy = nc.dram_tensor("sy", [32, RS], F16, kind="ExternalInput")
    la = nc.dram_tensor("la", [12, 128], BF16, kind="ExternalInput")
    lb1 = nc.dram_tensor("lb1", [P, 64], F16, kind="ExternalInput")
    lb2 = nc.dram_tensor("lb2", [64, ZC * 32], F16, kind="ExternalInput")
    c20 = nc.dram_tensor("c20", [P, 1], F32, kind="ExternalInput")
    zs = nc.dram_tensor("zs", [ZO * ZC, SF], F32, kind="ExternalOutput")

    x3 = xt.ap().rearrange("(pp r) d -> pp (r d)", pp=P)   # [P, RPPG*D]
    y2 = y.ap().rearrange("(pp r) -> pp r", pp=P)
    z2 = z.ap().rearrange("(pp r) -> pp r", pp=P)

    with TileContext(nc) as tc:
        with (
            tc.tile_pool(name="const", bufs=1) as cpool,
            tc.tile_pool(name="sb", bufs=4) as pool,
            tc.tile_pool(name="gat", bufs=4) as gpool,
            tc.tile_pool(name="sp", bufs=3) as spool,
            tc.tile_pool(name="gps", bufs=1, space="PSUM") as gppool,
            tc.tile_pool(name="apsp", bufs=2, space="PSUM") as appool,
            tc.tile_pool(name="tpsp", bufs=2, space="PSUM") as tppool,
            tc.tile_pool(name="zp", bufs=2, space="PSUM") as zpool,
        ):
            wb_t = cpool.tile([P, C], F32)
            nc.sync.dma_start(out=wb_t[:], in_=wb[:, :])
            pw_t = cpool.tile([P, D], F16)
            nc.sync.dma_start(out=pw_t[:], in_=pw[:, :])
            mk_t = cpool.tile([P, 16 * P], BF16)
            nc.sync.dma_start(out=mk_t[:], in_=mk[:, :])
            la_t = cpool.tile([12, 128], BF16)
            nc.sync.dma_start(out=la_t[:], in_=la[:, :])
            lb1_t = cpool.tile([P, 64], F16)
            nc.sync.dma_start(out=lb1_t[:], in_=lb1[:, :])
            lb2_t = cpool.tile([64, ZC * 32], F16)
            nc.sync.dma_start(out=lb2_t[:], in_=lb2[:, :])
            c20_t = cpool.tile([P, 1], F32)
            nc.sync.dma_start(out=c20_t[:], in_=c20[:, :])

            # ---------------- gather path helpers ----------------
            def g_load_idx(i, off, B):
                xtt = pool.tile([P, B * D], F16, tag="x")
                nc.sync.dma_start(out=xtt[:], in_=x3[:, off * D:(off + B) * D])
                yt = pool.tile([P, B], F32, tag="y")
                nc.sync.dma_start(out=yt[:], in_=y2[:, off:off + B])
                xv = xtt[:].rearrange("p (b d) -> p b d", d=D)
                nc.vector.tensor_tensor(
                    out=xv, in0=xv,
                    in1=pw_t[:].unsqueeze(1).broadcast_to([P, B, D]),
                    op=mybir.AluOpType.mult,
                )
                idxf = pool.tile([P, B], F16, tag="idxf")
                with nc.allow_low_precision(
                    reason="index accumulation is integer-exact in fp16 (<=2047)"
                ):
                    nc.vector.tensor_reduce(
                        out=idxf[:], in_=xv, axis=mybir.AxisListType.X,
                        op=mybir.AluOpType.add,
                    )
                idx16 = pool.tile([P, B], I16, tag="idx16")
                nc.vector.tensor_copy(out=idx16[:], in_=idxf[:])
                return yt, idx16

            def g_gather(idx16, B):
                og = gpool.tile([P, 16 * B], F32, tag="og")
                nc.gpsimd.ap_gather(
                    out_ap=og[:].rearrange("p (j e) -> p j e", e=1),
                    in_ap=wb_t[:].rearrange("p (c e) -> p c e", e=1),
                    idxs_ap=idx16[:],
                    channels=P, num_elems=C, d=1, num_idxs=16 * B,
                )
                return og

            def g_unwrap(og, yt, off, B):
                og3 = og[:].bitcast(BF16).rearrange("p (c s) -> p c s", s=32)
                psw = gppool.tile([P, B], F32, tag="psw")
                psb = gppool.tile([P, B], F32, tag="psb")
                for q in range(16):
                    nc.tensor.matmul(
                        out=psw[:], lhsT=mk_t[:, q * P:(q + 1) * P],
                        rhs=og3[:, :, 2 * q], start=(q == 0), stop=(q == 15),
                    )
                for q in range(16):
                    nc.tensor.matmul(
                        out=psb[:], lhsT=mk_t[:, q * P:(q + 1) * P],
                        rhs=og3[:, :, 2 * q + 1], start=(q == 0), stop=(q == 15),
                    )
                zt = pool.tile([P, B], F32, tag="z")
                nc.vector.tensor_tensor(
                    out=zt[:], in0=yt[:], in1=psw[:], op=mybir.AluOpType.mult
                )
                nc.vector.tensor_tensor(
                    out=zt[:], in0=zt[:], in1=psb[:], op=mybir.AluOpType.add
                )
                nc.sync.dma_start(out=z2[:, off:off + B], in_=zt[:])

            # ---------------- side path (skew-2 software pipeline) ----------------
            SBAT = 8                        # subtiles per batched DMA
            zstate = {"zps": None}
            st = {"bits": {}, "yr": {}, "sg": {}, "pr": {}, "aps": {}, "tps": {}}

            def s_batch_dma(k):
                c0 = k * SBAT * SF
                bb = spool.tile([12, SBAT * SF], BF16, tag="bits", name="bb")
                nc.sync.dma_start(out=bb[:], in_=sb[:, c0:c0 + SBAT * SF])
                yy = spool.tile([32, SBAT * SF], F16, tag="yr", name="yy")
                nc.sync.dma_start(out=yy[:], in_=sy[:, c0:c0 + SBAT * SF])
                st["bits"][k] = bb
                st["yr"][k] = yy

            def s_stage_a(s):
                k, r = divmod(s, SBAT)
                if r == 0 and k + 1 < ZO * ZC // SBAT:
                    s_batch_dma(k + 1)
                bits = st["bits"][k][:, r * SF:(r + 1) * SF]
                aps = appool.tile([P, SF], F32, tag="aps", name="aps")
                nc.tensor.matmul(out=aps[:], lhsT=la_t[:], rhs=bits,
                                 start=True, stop=True)
                st["aps"][s] = aps

            def s_stage_sg(s):
                k, r = divmod(s, SBAT)
                # steps: exact 0/1 for integer args (sigma(+->=20) saturates)
                sg = spool.tile([P, SF], F16, tag="sg", name="sg")
                nc.scalar.activation(
                    out=sg[:], in_=st["aps"].pop(s)[:],
                    func=mybir.ActivationFunctionType.Sigmoid,
                    bias=c20_t[:], scale=40.0,
                )
                # scale w-plane lo-steps by y
                nc.vector.tensor_tensor(
                    out=sg[0:32, :], in0=sg[0:32, :],
                    in1=st["yr"][k][:, r * SF:(r + 1) * SF],
                    op=mybir.AluOpType.mult,
                )
                st["sg"][s] = sg

            def s_stage_b1(s):
                sg = st["sg"][s]
                tps = tppool.tile([64, SF], F32, tag="tps", name="tps")
                nc.tensor.matmul(out=tps[:], lhsT=lb1_t[64:128, :],
                                 rhs=sg[64:128, :], start=True, stop=True)
                st["tps"][s] = tps

            def s_stage_pr(s):
                sg = st["sg"].pop(s)
                pr = spool.tile([64, SF], F16, tag="pr", name="pr")
                nc.vector.tensor_tensor(
                    out=pr[:], in0=st["tps"].pop(s)[:], in1=sg[0:64, :],
                    op=mybir.AluOpType.mult,
                )
                st["pr"][s] = pr

            def s_stage_b2(s):
                o, j = divmod(s, ZC)
                if j == 0:
                    zstate["zps"] = zpool.tile([ZC, SF], F32, tag="zps", name="zps")
                nc.tensor.matmul(
                    out=zstate["zps"][:], lhsT=lb2_t[:, 32 * j:32 * (j + 1)],
                    rhs=st["pr"].pop(s)[:], start=(j == 0), stop=(j == ZC - 1),
                )
                if j == ZC - 1:
                    zsb = spool.tile([ZC, SF], F32, tag="zsb", name="zsb")
                    nc.vector.tensor_copy(out=zsb[:], in_=zstate["zps"][:])
                    nc.sync.dma_start(out=zs[o * ZC:(o + 1) * ZC, :], in_=zsb[:])

            def s_subtile(it, total):
                # iteration it: A(it), B1(it-1), B2(it-2) keep PE dense
                if it < total:
                    s_stage_a(it)
                if 0 <= it - 1 < total:
                    s_stage_b1(it - 1)
                if 0 <= it - 2 < total:
                    s_stage_b2(it - 2)
                if it < total:
                    s_stage_sg(it)
                if 0 <= it - 1 < total:
                    s_stage_pr(it - 1)

            # ---------------- emission: interleave rounds ----------------
            nrounds = len(G_SCHED)
            goff = [0]
            for B in G_SCHED:
                goff.append(goff[-1] + B)
            gpre = [g_load_idx(i, goff[i], G_SCHED[i]) for i in range(nrounds)]
            ogs = [g_gather(gpre[i][1], G_SCHED[i]) for i in range(nrounds)]
            s_batch_dma(0)
            total = ZO * ZC
            sctr = 0
            for i in range(nrounds):
                for _ in range(SUBS_PER_ROUND):
                    s_subtile(sctr, total)
                    sctr += 1
                if i >= 1:
                    with tc.tile_wait_until(0.05 + 0.05 * (i - 1)):
                        g_unwrap(ogs[i - 1], gpre[i - 1][0], goff[i - 1],
                                 G_SCHED[i - 1])
            while sctr < total + 2:
                s_subtile(sctr, total)
                sctr += 1
            with tc.tile_wait_until(0.05 + 0.05 * (len(G_SCHED) - 1)):
                g_unwrap(ogs[-1], gpre[-1][0], goff[len(G_SCHED) - 1],
                         G_SCHED[-1])

    nc.compile()
    return nc


def _get_program():
    if "nc" not in _CACHE:
        _CACHE["nc"] = _build_program()
    return _CACHE["nc"]


def _host_prep(x, t, y, w, b):
    N = x.shape[0]
    npad = M * R - N
    assert npad >= 0
    f32 = np.float32
    f16 = np.float16
    bf16 = ml_dtypes.bfloat16

    # full [t|x] bit matrix [M*R, 11] fp16 (values 0/1 exact)
    bits = np.zeros((M * R, D), f16)
    bits[:N, 0] = np.asarray(t, f32).reshape(-1)
    bits[:N, 1:] = np.asarray(x, f32)
    yfull = np.concatenate([np.asarray(y, f32).reshape(-1), np.zeros(npad, f32)])

    bits_c = bits.reshape(M, R, D)
    y_c = yfull.reshape(M, R)

    # gather path: first RG rows of each shard, row-major per partition
    xtp = np.ascontiguousarray(bits_c[:, :RG, :])                    # [M, RG, 11]
    yp = np.ascontiguousarray(y_c[:, :RG])                           # [M, RG]
    # side path: remaining RS rows, column-major [12, RS] with ones row
    sbp = np.empty((M, 12, RS), ml_dtypes.bfloat16)
    sbp[:, :D, :] = bits_c[:, RG:, :].transpose(0, 2, 1)
    sbp[:, D, :] = 1.0
    syp = np.ascontiguousarray(
        np.broadcast_to(y_c[:, None, RG:], (M, 32, RS)).astype(f16)
    )

    # constants
    wbits = np.asarray(w, f32).astype(bf16).view(np.uint16).astype(np.uint32)
    bbits = np.asarray(b, f32).astype(bf16).view(np.uint16).astype(np.uint32)
    wbi = (wbits | (bbits << 16)).view(f32)
    wb_rep = np.ascontiguousarray(np.tile(wbi[None, :], (P, 1)))
    pw_rep = np.ascontiguousarray(
        np.tile((2.0 ** np.arange(D - 1, -1, -1)).astype(f32)[None, :], (P, 1))
    ).astype(f16)
    mk_host = np.zeros((P, 16 * P), f32)
    for k in range(P):
        mk_host[k, (k % 16) * P + k] = 1.0
    mk_host = mk_host.astype(bf16)

    # side-path weights
    la_h = np.zeros((12, 128), f32)
    hi_pw = np.array([32, 16, 8, 4, 2, 1], f32)       # t, x0..x4
    lo_pw = np.array([16, 8, 4, 2, 1], f32)           # x5..x9
    for m in range(64):
        l = m % 32
        la_h[6:11, m] = lo_pw
        la_h[11, m] = -float(l)
    for c in range(63):
        la_h[0:6, 64 + c] = hi_pw
        la_h[11, 64 + c] = -float(c + 1)
    la_h[11, 127] = 1.0

    W2 = np.asarray(w, f32).reshape(64, 32)
    B2t = np.asarray(b, f32).reshape(64, 32)

    def dd(T):
        G = np.empty_like(T)
        G[:63] = T[1:] - T[:-1]
        G[63] = T[0]
        Gd = np.empty_like(G)
        Gd[:, 0] = G[:, 0]
        Gd[:, 1:] = G[:, 1:] - G[:, :-1]
        return Gd

    lb1_h = np.zeros((P, 64), f32)
    lb1_h[64:, :32] = dd(W2)
    lb1_h[64:, 32:] = dd(B2t)

    lb2_h = np.zeros((64, ZC * 32), f32)
    for j in range(ZC):
        lb2_h[:, 32 * j + j] = 1.0

    in_maps = []
    for i in range(M):
        in_maps.append({
            "xt": xtp[i], "y": yp[i], "wb": wb_rep, "pw": pw_rep,
            "mk": mk_host, "sb": np.ascontiguousarray(sbp[i]),
            "sy": syp[i], "la": la_h.astype(ml_dtypes.bfloat16),
            "lb1": lb1_h.astype(f16), "lb2": lb2_h.astype(f16),
            "c20": np.full((P, 1), 20.0, f32),
        })
    return in_maps, npad


def kernel(x, t, y, w, b, trace=False):
    N = x.shape[0]
    in_maps, npad = _host_prep(x, t, y, w, b)
    nc = _get_program()
    res = run_bass_kernel_spmd(nc, in_maps, core_ids=list(range(M)), trace=trace)
    out = np.empty((M, R), np.float32)
    for i in range(M):
        out[i, :RG] = res.results[i]["z"]
        out[i, RG:] = res.results[i]["zs"].reshape(-1)
    zfull = out.reshape(-1)[:N].reshape(N, 1).astype(np.float32)
    if trace:
        return zfull, res
    return zfull
